# revision 7
# baseline (speedup 1.0000x reference)
"""AttentionBlock kernel for 8 Trainium2 NeuronCores.

Reference op: GroupNorm(8 groups) -> 1x1 conv qkv -> 8-head attention over
1024 spatial positions -> 1x1 conv proj -> residual.   Shapes (full):
x [8, 512, 32, 32]; qkv_w [1536, 512]; proj_w [512, 512].

Sharding: pure data-parallel over batch - one batch element per core.

Design notes (v2, fp8 DoubleRow):
  - PE matmul cost is output-columns x cycles/row, independent of the
    contraction depth; fp8 DoubleRow (0.5 cyc/row) doubles throughput by
    packing two 128-deep contraction groups per instruction.  qkv, scores,
    AV and proj all run as fp8e4 DR; weights are scaled x32 into fp8 range
    (TRN e4m3 max is +-240) and unscaled at PSUM eviction.
  - Scores per head contract over d=64, split as two DR groups of 32; q/k
    are evicted to fp8 then DMA-shuffled into [32, 2, n] pair layout.
  - exp() runs with a 2^-7 shift folded into its bias so probs fit fp8;
    softmax is shift-invariant.  The exp work is split between the ACT
    engine (exact spline exp) and the DVE (fp32->uint8 bit-trick that
    directly materializes e4m3-encoded 2^y, ~5% per-element, diluted by
    mixing with exact tiles across m-chunks).
  - Denominators come from a dedicated ones-matmul over the fp8 prob tiles
    (accumulated into one [8, n] PSUM tile via one-hot stationaries) so
    reciprocals are ready before AV completes; 1/d = exp(-ln d) on ACT
    reuses the already-loaded natural_log_exp table.
  - The division is fused into the AV PSUM eviction (one tensor_tensor
    against a DMA-broadcast reciprocal tile).
  - v-bias folds through softmax+proj into a host-side residual term
    (xpb = x + proj_b + proj_w @ v_bias); q/k biases fold into the
    per-partition bias of their PSUM evictions.
"""

import os

import numpy as np
import ml_dtypes

NCORES = 8
C = 512
N = 1024  # 32*32 spatial
NH = 8
HD = 64  # head dim
CCH = 4  # channel chunks of 128
EPS = 1e-5
WS = 32.0  # fp8 weight scale
ESH = 7.0  # exp shift (probs scaled 2^-ESH)
LOG2E = 1.4426950408889634

# exp engine split, pattern over (h, mb) tiles with period 8:
#   'A' = ACT exact spline exp -> fp8
#   'U' = DVE fp32->uint8 bit-trick (1 op, ~5%/elem, keep rare)
#   'C' = DVE fp32->int16 bf16 bit-trick + Pool bf16->fp8 convert (accurate)
EXP_PATTERN = os.environ.get("EXP_PATTERN", "AACAUACA")
BITEXP_CORR = 0.0437  # linear-interp exp2 centering

_CACHE = {}
LAST = {"exec_time_ns": None, "results": None}


def _build_program():
    import concourse.bass as bass
    import concourse.tile as tile
    from concourse import mybir

    f32 = mybir.dt.float32
    bf16 = mybir.dt.bfloat16
    f8 = mybir.dt.float8e4
    u8 = mybir.dt.uint8
    i16 = mybir.dt.int16
    AF = mybir.ActivationFunctionType
    OP = mybir.AluOpType
    DR = mybir.MatmulPerfMode.DoubleRow

    nc = bass.Bass()

    # ---- DRAM parameters (per core). Host pre-reshapes/pre-scales. ----
    x_d = nc.declare_dram_parameter("x", [CCH, 128, N], f32, isOutput=False)
    xpb_d = nc.declare_dram_parameter("xpb", [CCH, 128, N], f32, isOutput=False)
    qkvw_d = nc.declare_dram_parameter("qkvw8", [2, 128, 2, 3 * C], f8, isOutput=False)
    pw_d = nc.declare_dram_parameter("pw8", [2, 128, 2, C], f8, isOutput=False)
    qb_d = nc.declare_dram_parameter("qb", [CCH, 128, 1], f32, isOutput=False)
    kb_d = nc.declare_dram_parameter("kb", [CCH, 128, 1], f32, isOutput=False)
    gnw_d = nc.declare_dram_parameter("gnw", [128, CCH], f32, isOutput=False)
    gnb_d = nc.declare_dram_parameter("gnb", [128, CCH], f32, isOutput=False)
    mask_d = nc.declare_dram_parameter("gn_mask", [128, 128], f32, isOutput=False)
    dsel_d = nc.declare_dram_parameter("dsel", [128, 2, NH, NH], f8, isOutput=False)
    out_d = nc.declare_dram_parameter("out", [CCH, 128, N], f32, isOutput=True)

    from contextlib import ExitStack

    with (
        nc.allow_low_precision(reason="fp8 double-row matmuls within tolerance"),
        tile.TileContext(nc) as tc,
        ExitStack() as ctx,
    ):
        consts = ctx.enter_context(tc.tile_pool(name="consts", bufs=1))
        xp = ctx.enter_context(tc.tile_pool(name="xp", bufs=1))
        xpbp = ctx.enter_context(tc.tile_pool(name="xpbp", bufs=1))
        wp = ctx.enter_context(tc.tile_pool(name="wp", bufs=1))
        xnp = ctx.enter_context(tc.tile_pool(name="xnp", bufs=1))
        qkp = ctx.enter_context(tc.tile_pool(name="qkp", bufs=1))
        pairp = ctx.enter_context(tc.tile_pool(name="pairp", bufs=1))
        vtp = ctx.enter_context(tc.tile_pool(name="vtp", bufs=1))
        ap_pool = ctx.enter_context(tc.tile_pool(name="ap", bufs=1))
        gnp = ctx.enter_context(tc.tile_pool(name="gnp", bufs=1))
        o8p = ctx.enter_context(tc.tile_pool(name="o8p", bufs=1))
        recp = ctx.enter_context(tc.tile_pool(name="recp", bufs=1))
        tmpp = ctx.enter_context(tc.tile_pool(name="tmpp", bufs=2))
        outp = ctx.enter_context(tc.tile_pool(name="outp", bufs=2))
        dramp = ctx.enter_context(tc.tile_pool(name="dramp", bufs=1, space="DRAM"))
        # PSUM: scores [128,1024] x2 (4 banks) + AV [64,512] x2 (2) + denom [8,1024] (2)
        ps_s = ctx.enter_context(tc.tile_pool(name="ps_s", bufs=2, space="PSUM"))
        ps_av = ctx.enter_context(tc.tile_pool(name="ps_av", bufs=2, space="PSUM"))
        ps_d = ctx.enter_context(tc.tile_pool(name="ps_d", bufs=1, space="PSUM"))

        # ---- constants ----
        mask_sb = consts.tile([128, 128], f32, tag="mask")
        nc.sync.dma_start(out=mask_sb, in_=mask_d[:, :])
        gnw_all = consts.tile([128, CCH], f32, tag="gnw")
        nc.sync.dma_start(out=gnw_all, in_=gnw_d[:, :])
        gnb_all = consts.tile([128, CCH], f32, tag="gnb")
        nc.sync.dma_start(out=gnb_all, in_=gnb_d[:, :])
        qb_sb = []
        kb_sb = []
        for cc in range(CCH):
            t = consts.tile([128, 1], f32, tag=f"qb{cc}")
            nc.sync.dma_start(out=t, in_=qb_d[cc])
            qb_sb.append(t)
            t = consts.tile([128, 1], f32, tag=f"kb{cc}")
            nc.sync.dma_start(out=t, in_=kb_d[cc])
            kb_sb.append(t)
        dsel_sb = consts.tile([128, 2, NH, NH], f8, tag="dsel")
        nc.sync.dma_start(out=dsel_sb, in_=dsel_d[:, :, :, :])
        eps_sb = consts.tile([128, 1], f32, tag="eps")
        nc.vector.memset(eps_sb, EPS)
        zero_sb = consts.tile([128, 1], f32, tag="zero")
        nc.vector.memset(zero_sb, 0.0)
        ebias_sb = consts.tile([128, 1], f32, tag="ebias")
        nc.vector.memset(ebias_sb, -ESH * 0.6931471805599453)

        # ---- load x ----
        x_sb = []
        for cc in range(CCH):
            t = xp.tile([128, N], f32, tag=f"x{cc}")
            nc.sync.dma_start(out=t, in_=x_d[cc])
            x_sb.append(t)

        # ---- weights ----
        qkvw_sb = []
        for kc in range(2):
            t = wp.tile([128, 2, 3 * C], f8, tag=f"qw{kc}")
            nc.sync.dma_start(out=t, in_=qkvw_d[kc])
            qkvw_sb.append(t)
        pw_sb = []
        for kc in range(2):
            t = wp.tile([128, 2, C], f8, tag=f"pw{kc}")
            nc.sync.dma_start(out=t, in_=pw_d[kc])
            pw_sb.append(t)

        # ---- GroupNorm stats (as v1: bn_stats + block-diag mask matmul) ----
        mv_all = gnp.tile([128, CCH, 2], f32, tag="mv")
        for cc in range(CCH):
            stats = gnp.tile([128, 2, 6], f32, tag=f"st{cc}")
            for sg in range(2):
                nc.vector.bn_stats(
                    out=stats[:, sg, :], in_=x_sb[cc][:, sg * 512 : (sg + 1) * 512]
                )
            nc.vector.bn_aggr(out=mv_all[:, cc, :], in_=stats)
        st2 = gnp.tile([128, CCH, 2], f32, tag="s2")
        nc.vector.tensor_copy(out=st2[:, :, 0], in_=mv_all[:, :, 0])
        mean_sq = gnp.tile([128, CCH], f32, tag="msq")
        nc.vector.tensor_mul(out=mean_sq, in0=mv_all[:, :, 0], in1=mv_all[:, :, 0])
        nc.vector.tensor_add(out=st2[:, :, 1], in0=mv_all[:, :, 1], in1=mean_sq)
        ps_gn = ps_s.tile([128, N], f32, tag="s")
        ps_st = ps_gn[:, 0 : CCH * 2]
        nc.tensor.matmul(
            ps_st,
            lhsT=mask_sb,
            rhs=st2.rearrange("p c two -> p (c two)"),
            start=True,
            stop=True,
        )
        gst = gnp.tile([128, CCH, 2], f32, tag="gst")
        nc.vector.tensor_copy(
            out=gst, in_=ps_st.rearrange("p (c two) -> p c two", two=2)
        )
        gm2 = gnp.tile([128, CCH], f32, tag="g2")
        nc.vector.tensor_mul(out=gm2, in0=gst[:, :, 0], in1=gst[:, :, 0])
        gvar = gnp.tile([128, CCH], f32, tag="gv")
        nc.vector.tensor_sub(out=gvar, in0=gst[:, :, 1], in1=gm2)
        # rstd = exp(-0.5*ln(var+eps)); warms the natural_log_exp ACT table.
        lnv = gnp.tile([128, CCH], f32, tag="lnv")
        nc.scalar.activation(out=lnv, in_=gvar, func=AF.Ln, bias=eps_sb)
        rstd = gnp.tile([128, CCH], f32, tag="rstd")
        nc.scalar.activation(out=rstd, in_=lnv, func=AF.Exp, scale=-0.5, bias=zero_sb)
        gscale = gnp.tile([128, CCH], f32, tag="gs")
        nc.vector.tensor_mul(out=gscale, in0=rstd, in1=gnw_all)
        t4 = gnp.tile([128, CCH], f32, tag="t4")
        nc.vector.tensor_mul(out=t4, in0=gst[:, :, 0], in1=gscale)
        gbias = gnp.tile([128, CCH], f32, tag="gb")
        nc.vector.tensor_sub(out=gbias, in0=gnb_all, in1=t4)

        # ---- xn in fp8, chunk-major layout [128, cc, N] (Pool engine) ----
        xn8 = xnp.tile([128, CCH, N], f8, tag="xn8")
        for cc in range(CCH):
            nc.gpsimd.tensor_scalar(
                out=xn8[:, cc, :],
                in0=x_sb[cc],
                scalar1=gscale[:, cc : cc + 1],
                scalar2=gbias[:, cc : cc + 1],
                op0=OP.mult,
                op1=OP.add,
            )

        # ---- qkv matmuls (fp8 DR) ----
        sc = float(HD**-0.5)
        # q/k chunks: stationary w pairs, moving xn pairs -> out [128ch, n];
        # per-oc ordering (q, k, then the DMA shuffles for heads 2oc/2oc+1)
        # so scores for early heads can start while later chunks compute.
        qp = pairp.tile([32, NH, 2, N], f8, tag="qp")
        kp = pairp.tile([32, NH, 2, N], f8, tag="kp")
        q8 = []
        k8 = []
        for oc in range(CCH):
            for which in ("q", "k"):
                base = 0 if which == "q" else C
                ps = ps_s.tile([128, N], f32, tag="s")
                for nj in range(2):
                    for kc in range(2):
                        nc.tensor.matmul(
                            ps[:, nj * 512 : (nj + 1) * 512],
                            lhsT=qkvw_sb[kc][:, :, base + oc * 128 : base + (oc + 1) * 128],
                            rhs=xn8[:, 2 * kc : 2 * kc + 2, nj * 512 : (nj + 1) * 512],
                            start=(kc == 0),
                            stop=(kc == 1),
                            perf_mode=DR,
                        )
                t = qkp.tile([128, N], f8, tag=f"{which}{oc}")
                if which == "q":
                    nc.scalar.activation(
                        out=t, in_=ps, func=AF.Identity, bias=qb_sb[oc], scale=sc / WS
                    )
                    q8.append(t)
                else:
                    nc.scalar.activation(
                        out=t, in_=ps, func=AF.Identity, bias=kb_sb[oc], scale=1.0 / WS
                    )
                    k8.append(t)
            for hh in range(2):
                h = 2 * oc + hh
                for i in range(2):
                    src_lo = 64 * hh + 32 * i
                    nc.sync.dma_start(
                        out=qp[:, h, i, :], in_=q8[oc][src_lo : src_lo + 32, :]
                    )
                    nc.sync.dma_start(
                        out=kp[:, h, i, :], in_=k8[oc][src_lo : src_lo + 32, :]
                    )

        # ---- scores + exp + denominators (+ vT blocks interleaved) ----
        a8 = ap_pool.tile([128, NH, NH, N], f8, tag="a8")  # (mb, h, n)
        vt8 = vtp.tile([128, NH, NH, HD], f8, tag="vt8")  # (mb, h, d)
        psD = ps_d.tile([8, N], f32, tag="den")
        exp_idx = 0

        def emit_scores(h):
            nonlocal exp_idx
            for mb in range(8):
                ps = ps_s.tile([128, N], f32, tag="s")
                for nj in range(2):
                    nc.tensor.matmul(
                        ps[:, nj * 512 : (nj + 1) * 512],
                        lhsT=kp[:, h, :, mb * 128 : (mb + 1) * 128],
                        rhs=qp[:, h, :, nj * 512 : (nj + 1) * 512],
                        start=True,
                        stop=True,
                        perf_mode=DR,
                    )
                dst = a8[:, mb, h, :]
                kind = EXP_PATTERN[exp_idx % len(EXP_PATTERN)]
                if kind == "A":
                    nc.scalar.activation(
                        out=dst, in_=ps, func=AF.Exp, bias=ebias_sb, scale=1.0
                    )
                elif kind == "U":
                    # i8 = round(S*8*log2e - corr*8); uint8 clamps both ends
                    nc.vector.tensor_scalar(
                        out=dst.bitcast(u8),
                        in0=ps,
                        scalar1=8.0 * LOG2E,
                        scalar2=-8.0 * BITEXP_CORR,
                        op0=OP.mult,
                        op1=OP.add,
                    )
                else:
                    # i16 = round(S*128*log2e + (127-ESH-corr)*128); bits = bf16 2^y
                    t16 = tmpp.tile([128, N], i16, tag="t16")
                    nc.vector.tensor_scalar(
                        out=t16,
                        in0=ps,
                        scalar1=128.0 * LOG2E,
                        scalar2=128.0 * (127.0 - ESH - BITEXP_CORR),
                        op0=OP.mult,
                        op1=OP.add,
                    )
                    nc.gpsimd.tensor_scalar(
                        out=dst,
                        in0=t16.bitcast(bf16),
                        scalar1=1.0,
                        scalar2=0.0,
                        op0=OP.mult,
                        op1=OP.add,
                    )
                exp_idx += 1

        def emit_vt(mb):
            ps = ps_s.tile([128, N], f32, tag="s")
            for kc in range(2):
                nc.tensor.matmul(
                    ps[:, 0:512],
                    lhsT=xn8[:, 2 * kc : 2 * kc + 2, mb * 128 : (mb + 1) * 128],
                    rhs=qkvw_sb[kc][:, :, 2 * C : 3 * C],
                    start=(kc == 0),
                    stop=(kc == 1),
                    perf_mode=DR,
                )
            nc.scalar.activation(
                out=vt8[:, mb, :, :],
                in_=ps[:, 0:512].rearrange("p (h d) -> p h d", h=NH),
                func=AF.Copy,
                scale=1.0 / WS,
            )

        def emit_denom(h):
            for j in range(4):
                for nj in range(2):
                    nc.tensor.matmul(
                        psD[:, nj * 512 : (nj + 1) * 512],
                        lhsT=dsel_sb[:, :, h, :],
                        rhs=a8[:, 2 * j : 2 * j + 2, h, nj * 512 : (nj + 1) * 512],
                        start=(h == 0 and j == 0),
                        stop=(h == 7 and j == 3),
                        perf_mode=DR,
                    )

        emit_scores(0)
        for mb in range(8):
            emit_vt(mb)
        for h in range(1, NH):
            emit_scores(h)
            emit_denom(h - 1)
        emit_denom(NH - 1)

        # ---- reciprocals: 1/d = exp(-ln d), broadcast via DRAM bounce ----
        lnd = recp.tile([8, N], f32, tag="lnd")
        nc.scalar.activation(out=lnd, in_=psD, func=AF.Ln, bias=zero_sb[0:8])
        recb = recp.tile([8, N], bf16, tag="recb")
        nc.scalar.activation(out=recb, in_=lnd, func=AF.Exp, scale=-1.0, bias=zero_sb[0:8])
        rec_dram = dramp.tile([8, N], bf16, tag="recd")
        nc.sync.dma_start(out=rec_dram, in_=recb)
        rec_b = recp.tile([64, NH, N], bf16, tag="recbc")
        for h in range(NH):
            row = rec_dram[h : h + 1, :]
            bcast = bass.AP(
                tensor=row.tensor,
                offset=row.offset,
                ap=[[0, 64]] + [list(x) for x in row.ap[1:]],
            )
            nc.sync.dma_start(out=rec_b[:, h, :], in_=bcast)

        # ---- AV (fp8 DR) + fused division eviction ----
        o8 = o8p.tile([128, 2, 2, N], f8, tag="o8")  # (kc, i, n); ch = kc*256+i*128+p
        for h in range(NH):
            for nj in range(2):
                av = ps_av.tile([64, 512], f32, tag="av")
                for j in range(4):
                    nc.tensor.matmul(
                        av,
                        lhsT=vt8[:, 2 * j : 2 * j + 2, h, :],
                        rhs=a8[:, 2 * j : 2 * j + 2, h, nj * 512 : (nj + 1) * 512],
                        start=(j == 0),
                        stop=(j == 3),
                        perf_mode=DR,
                    )
                nc.vector.tensor_tensor(
                    out=o8[
                        64 * (h % 2) : 64 * (h % 2) + 64,
                        h // 4,
                        (h % 4) // 2,
                        nj * 512 : (nj + 1) * 512,
                    ],
                    in0=av,
                    in1=rec_b[:, h, nj * 512 : (nj + 1) * 512],
                    op=OP.mult,
                )

        # ---- xpb (residual + folded proj/v biases) ----
        xpb_sb = []
        for cc in range(CCH):
            t = xpbp.tile([128, N], f32, tag=f"xpb{cc}")
            nc.sync.dma_start(out=t, in_=xpb_d[cc])
            xpb_sb.append(t)

        # ---- proj (fp8 DR) + residual ----
        for oc in range(CCH):
            ot = outp.tile([128, N], f32, tag="ot")
            for nj in range(2):
                ps = ps_s.tile([128, N], f32, tag="s")
                for kc in range(2):
                    nc.tensor.matmul(
                        ps[:, nj * 512 : (nj + 1) * 512],
                        lhsT=pw_sb[kc][:, :, oc * 128 : (oc + 1) * 128],
                        rhs=o8[:, kc, :, nj * 512 : (nj + 1) * 512],
                        start=(kc == 0),
                        stop=(kc == 1),
                        perf_mode=DR,
                    )
                nc.vector.scalar_tensor_tensor(
                    out=ot[:, nj * 512 : (nj + 1) * 512],
                    in0=ps[:, nj * 512 : (nj + 1) * 512],
                    scalar=1.0 / WS,
                    in1=xpb_sb[oc][:, nj * 512 : (nj + 1) * 512],
                    op0=OP.mult,
                    op1=OP.add,
                )
            nc.sync.dma_start(out=out_d[oc], in_=ot)

    _split_lw_waits(nc)
    return nc


def _split_lw_waits(nc):
    """This walrus build accepts only ONE sync-wait command per engine
    instruction; Tile can attach several. Hoist each excess wait onto its own
    pure sem-wait instruction inserted just before, in queue order."""
    from concourse import mybir

    for blk in nc.m.functions[0].blocks:
        out = []
        for inst in blk.instructions:
            si = getattr(inst, "sync_info", None)
            if (
                si is not None
                and si.on_wait
                and len(si.on_wait) > 1
                and type(inst).__name__ != "InstEventSemaphore"
            ):
                waits = list(si.on_wait)
                for j, w in enumerate(waits[:-1]):
                    sem = mybir.InstEventSemaphore(
                        name=f"{inst.name}_wsplit{j}",
                        engine=inst.engine,
                        ins=[],
                        outs=[],
                        sync_info=mybir.SyncInfo(on_wait=[w], on_update=[]),
                    )
                    out.append(sem)
                inst.sync_info = mybir.SyncInfo(
                    on_wait=waits[-1:], on_update=list(si.on_update or [])
                )
            out.append(inst)
        blk.instructions = out


def _f8(x, s=1.0):
    return np.ascontiguousarray(
        np.clip(x * s, -240.0, 240.0).astype(ml_dtypes.float8_e4m3)
    )


def kernel(x, gn_w, gn_b, qkv_w, qkv_b, proj_w, proj_b):
    from concourse.bass_utils import run_bass_kernel_spmd

    B = x.shape[0]
    assert B == NCORES
    if "nc" not in _CACHE:
        _CACHE["nc"] = _build_program()
    nc = _CACHE["nc"]

    x = np.asarray(x, dtype=np.float32)
    qkv_w = np.asarray(qkv_w, dtype=np.float32)
    qkv_b = np.asarray(qkv_b, dtype=np.float32)
    proj_w = np.asarray(proj_w, dtype=np.float32)
    proj_b = np.asarray(proj_b, dtype=np.float32)

    xf = np.ascontiguousarray(x.reshape(B, CCH, 128, N))
    # qkv weights in DR layout: [kc, cin128, i, cout], cin = kc*256+i*128+p
    wt = qkv_w.T.reshape(2, 2, 128, 3 * C)  # [kc, i, p, cout]
    qkvw8 = _f8(np.transpose(wt, (0, 2, 1, 3)), WS)  # [kc, p, i, cout]
    pwt = proj_w.T.reshape(2, 2, 128, C)
    pw8 = _f8(np.transpose(pwt, (0, 2, 1, 3)), WS)
    sc = HD**-0.5
    qb = np.ascontiguousarray((qkv_b[0:C] * sc).reshape(CCH, 128, 1))
    kb = np.ascontiguousarray(qkv_b[C : 2 * C].reshape(CCH, 128, 1))
    vb = qkv_b[2 * C : 3 * C]
    pb_eff = proj_b + proj_w @ vb
    xpb = np.ascontiguousarray(
        xf + pb_eff.reshape(CCH, 128, 1)[None, :, :, :]
    )
    gnw = np.ascontiguousarray(np.asarray(gn_w, np.float32).reshape(CCH, 128).T)
    gnb = np.ascontiguousarray(np.asarray(gn_b, np.float32).reshape(CCH, 128).T)
    mask = np.zeros((128, 128), dtype=np.float32)
    for g in range(2):
        mask[g * 64 : (g + 1) * 64, g * 64 : (g + 1) * 64] = 1.0 / 64.0
    dsel = np.zeros((128, 2, NH, NH), dtype=np.float32)
    for h in range(NH):
        dsel[:, :, h, h] = 1.0
    dsel = _f8(dsel)

    in_maps = []
    for i in range(NCORES):
        in_maps.append(
            {
                "x": xf[i],
                "xpb": xpb[i],
                "qkvw8": qkvw8,
                "pw8": pw8,
                "qb": qb,
                "kb": kb,
                "gnw": gnw,
                "gnb": gnb,
                "gn_mask": mask,
                "dsel": dsel,
            }
        )

    tmpdir = os.environ.get("BASS_TMPDIR")
    if tmpdir:
        os.makedirs(tmpdir, exist_ok=True)
    res = run_bass_kernel_spmd(
        nc,
        in_maps,
        list(range(NCORES)),
        trace=bool(os.environ.get("BASS_TRACE")),
        tmpdir=tmpdir,
    )
    LAST["exec_time_ns"] = res.exec_time_ns
    LAST["results"] = res
    out = np.stack([res.results[i]["out"] for i in range(NCORES)], axis=0)
    return out.reshape(B, C, 32, 32).astype(x.dtype)


# revision 10
# speedup vs baseline: 1.2257x; 1.2257x over previous
"""AttentionBlock kernel for 8 Trainium2 NeuronCores.

Reference op: GroupNorm(8 groups) -> 1x1 conv qkv -> 8-head attention over
1024 spatial positions -> 1x1 conv proj -> residual.   Shapes (full):
x [8, 512, 32, 32]; qkv_w [1536, 512]; proj_w [512, 512].

Sharding: pure data-parallel over batch - one batch element per core.

Design notes (v2, fp8 DoubleRow):
  - PE matmul cost is output-columns x cycles/row, independent of the
    contraction depth; fp8 DoubleRow (0.5 cyc/row) doubles throughput by
    packing two 128-deep contraction groups per instruction.  qkv, scores,
    AV and proj all run as fp8e4 DR; weights are scaled x32 into fp8 range
    (TRN e4m3 max is +-240) and unscaled at PSUM eviction.
  - Scores per head contract over d=64, split as two DR groups of 32; q/k
    are evicted to fp8 then DMA-shuffled into [32, 2, n] pair layout.
  - exp() runs with a 2^-7 shift folded into its bias so probs fit fp8;
    softmax is shift-invariant.  The exp work is split between the ACT
    engine (exact spline exp) and the DVE (fp32->uint8 bit-trick that
    directly materializes e4m3-encoded 2^y, ~5% per-element, diluted by
    mixing with exact tiles across m-chunks).
  - Denominators come from a dedicated ones-matmul over the fp8 prob tiles
    (accumulated into one [8, n] PSUM tile via one-hot stationaries) so
    reciprocals are ready before AV completes; 1/d = exp(-ln d) on ACT
    reuses the already-loaded natural_log_exp table.
  - The division is fused into the AV PSUM eviction (one tensor_tensor
    against a DMA-broadcast reciprocal tile).
  - v-bias folds through softmax+proj into a host-side residual term
    (xpb = x + proj_b + proj_w @ v_bias); q/k biases fold into the
    per-partition bias of their PSUM evictions.
"""

import os

import numpy as np
import ml_dtypes

NCORES = 8
C = 512
N = 1024  # 32*32 spatial
NH = 8
HD = 64  # head dim
CCH = 4  # channel chunks of 128
EPS = 1e-5
WS = 32.0  # fp8 weight scale
ESH = 7.0  # exp shift (probs scaled 2^-ESH)
LOG2E = 1.4426950408889634

# exp engine split, pattern over (h, mb) tiles with period 8:
#   'A' = ACT exact spline exp -> fp8
#   'U' = DVE fp32->uint8 bit-trick (1 op, ~5%/elem, keep rare)
#   'C' = DVE fp32->int16 bf16 bit-trick + Pool bf16->fp8 convert (accurate)
EXP_PATTERN = os.environ.get("EXP_PATTERN", "AACAUACA")
BITEXP_CORR = 0.0437  # linear-interp exp2 centering

_CACHE = {}
LAST = {"exec_time_ns": None, "results": None}


def _build_program():
    import concourse.bass as bass
    import concourse.tile as tile
    from concourse import mybir

    f32 = mybir.dt.float32
    bf16 = mybir.dt.bfloat16
    f8 = mybir.dt.float8e4
    u8 = mybir.dt.uint8
    i16 = mybir.dt.int16
    AF = mybir.ActivationFunctionType
    OP = mybir.AluOpType
    DR = mybir.MatmulPerfMode.DoubleRow

    nc = bass.Bass()

    # ---- DRAM parameters (per core). Host pre-reshapes/pre-scales. ----
    x_d = nc.declare_dram_parameter("x", [CCH, 128, N], f32, isOutput=False)
    xpb_d = nc.declare_dram_parameter("xpb", [CCH, 128, N], f32, isOutput=False)
    qkvw_d = nc.declare_dram_parameter("qkvw8", [2, 128, 2, 3 * C], f8, isOutput=False)
    pw_d = nc.declare_dram_parameter("pw8", [2, 128, 2, C], f8, isOutput=False)
    qb_d = nc.declare_dram_parameter("qb", [CCH, 128, 1], f32, isOutput=False)
    kb_d = nc.declare_dram_parameter("kb", [CCH, 128, 1], f32, isOutput=False)
    gnw_d = nc.declare_dram_parameter("gnw", [128, CCH], f32, isOutput=False)
    gnb_d = nc.declare_dram_parameter("gnb", [128, CCH], f32, isOutput=False)
    mask_d = nc.declare_dram_parameter("gn_mask", [128, 128], f32, isOutput=False)
    dsel_d = nc.declare_dram_parameter("dsel", [128, 2, NH, NH], f8, isOutput=False)
    out_d = nc.declare_dram_parameter("out", [CCH, 128, N], f32, isOutput=True)

    from contextlib import ExitStack

    with (
        nc.allow_low_precision(reason="fp8 double-row matmuls within tolerance"),
        tile.TileContext(nc) as tc,
        ExitStack() as ctx,
    ):
        consts = ctx.enter_context(tc.tile_pool(name="consts", bufs=1))
        xp = ctx.enter_context(tc.tile_pool(name="xp", bufs=1))
        xpbp = ctx.enter_context(tc.tile_pool(name="xpbp", bufs=1))
        wp = ctx.enter_context(tc.tile_pool(name="wp", bufs=1))
        xnp = ctx.enter_context(tc.tile_pool(name="xnp", bufs=1))
        qkp = ctx.enter_context(tc.tile_pool(name="qkp", bufs=1))
        vtp = ctx.enter_context(tc.tile_pool(name="vtp", bufs=1))
        ap_pool = ctx.enter_context(tc.tile_pool(name="ap", bufs=1))
        gnp = ctx.enter_context(tc.tile_pool(name="gnp", bufs=1))
        o8p = ctx.enter_context(tc.tile_pool(name="o8p", bufs=1))
        recp = ctx.enter_context(tc.tile_pool(name="recp", bufs=1))
        tmpp = ctx.enter_context(tc.tile_pool(name="tmpp", bufs=2))
        outp = ctx.enter_context(tc.tile_pool(name="outp", bufs=2))
        dramp = ctx.enter_context(tc.tile_pool(name="dramp", bufs=1, space="DRAM"))
        # PSUM: scores [128,1024] x2 (4 banks) + AV [64,512] x2 (2) + denom [8,1024] (2)
        ps_s = ctx.enter_context(tc.tile_pool(name="ps_s", bufs=2, space="PSUM"))
        ps_av = ctx.enter_context(tc.tile_pool(name="ps_av", bufs=2, space="PSUM"))
        ps_d = ctx.enter_context(tc.tile_pool(name="ps_d", bufs=1, space="PSUM"))

        # ---- constants ----
        mask_sb = consts.tile([128, 128], f32, tag="mask")
        nc.sync.dma_start(out=mask_sb, in_=mask_d[:, :])
        gnw_all = consts.tile([128, CCH], f32, tag="gnw")
        nc.sync.dma_start(out=gnw_all, in_=gnw_d[:, :])
        gnb_all = consts.tile([128, CCH], f32, tag="gnb")
        nc.sync.dma_start(out=gnb_all, in_=gnb_d[:, :])
        qb_sb = []
        kb_sb = []
        for cc in range(CCH):
            t = consts.tile([128, 1], f32, tag=f"qb{cc}")
            nc.sync.dma_start(out=t, in_=qb_d[cc])
            qb_sb.append(t)
            t = consts.tile([128, 1], f32, tag=f"kb{cc}")
            nc.sync.dma_start(out=t, in_=kb_d[cc])
            kb_sb.append(t)
        dsel_sb = consts.tile([128, 2, NH, NH], f8, tag="dsel")
        nc.sync.dma_start(out=dsel_sb, in_=dsel_d[:, :, :, :])
        eps_sb = consts.tile([128, 1], f32, tag="eps")
        nc.vector.memset(eps_sb, EPS)
        zero_sb = consts.tile([128, 1], f32, tag="zero")
        nc.vector.memset(zero_sb, 0.0)
        ebias_sb = consts.tile([128, 1], f32, tag="ebias")
        nc.vector.memset(ebias_sb, -ESH * 0.6931471805599453)

        # ---- load x ----
        x_sb = []
        for cc in range(CCH):
            t = xp.tile([128, N], f32, tag=f"x{cc}")
            nc.sync.dma_start(out=t, in_=x_d[cc])
            x_sb.append(t)

        # ---- weights ----
        qkvw_sb = []
        for kc in range(2):
            t = wp.tile([128, 2, 3 * C], f8, tag=f"qw{kc}")
            nc.sync.dma_start(out=t, in_=qkvw_d[kc])
            qkvw_sb.append(t)
        pw_sb = []
        for kc in range(2):
            t = wp.tile([128, 2, C], f8, tag=f"pw{kc}")
            nc.sync.dma_start(out=t, in_=pw_d[kc])
            pw_sb.append(t)

        # ---- GroupNorm stats (as v1: bn_stats + block-diag mask matmul) ----
        mv_all = gnp.tile([128, CCH, 2], f32, tag="mv")
        for cc in range(CCH):
            stats = gnp.tile([128, 2, 6], f32, tag=f"st{cc}")
            for sg in range(2):
                nc.vector.bn_stats(
                    out=stats[:, sg, :], in_=x_sb[cc][:, sg * 512 : (sg + 1) * 512]
                )
            nc.vector.bn_aggr(out=mv_all[:, cc, :], in_=stats)
        st2 = gnp.tile([128, CCH, 2], f32, tag="s2")
        nc.vector.tensor_copy(out=st2[:, :, 0], in_=mv_all[:, :, 0])
        mean_sq = gnp.tile([128, CCH], f32, tag="msq")
        nc.vector.tensor_mul(out=mean_sq, in0=mv_all[:, :, 0], in1=mv_all[:, :, 0])
        nc.vector.tensor_add(out=st2[:, :, 1], in0=mv_all[:, :, 1], in1=mean_sq)
        ps_gn = ps_s.tile([128, N], f32, tag="s")
        ps_st = ps_gn[:, 0 : CCH * 2]
        nc.tensor.matmul(
            ps_st,
            lhsT=mask_sb,
            rhs=st2.rearrange("p c two -> p (c two)"),
            start=True,
            stop=True,
        )
        gst = gnp.tile([128, CCH, 2], f32, tag="gst")
        nc.vector.tensor_copy(
            out=gst, in_=ps_st.rearrange("p (c two) -> p c two", two=2)
        )
        gm2 = gnp.tile([128, CCH], f32, tag="g2")
        nc.vector.tensor_mul(out=gm2, in0=gst[:, :, 0], in1=gst[:, :, 0])
        gvar = gnp.tile([128, CCH], f32, tag="gv")
        nc.vector.tensor_sub(out=gvar, in0=gst[:, :, 1], in1=gm2)
        # rstd = exp(-0.5*ln(var+eps)); warms the natural_log_exp ACT table.
        lnv = gnp.tile([128, CCH], f32, tag="lnv")
        nc.scalar.activation(out=lnv, in_=gvar, func=AF.Ln, bias=eps_sb)
        rstd = gnp.tile([128, CCH], f32, tag="rstd")
        nc.scalar.activation(out=rstd, in_=lnv, func=AF.Exp, scale=-0.5, bias=zero_sb)
        gscale = gnp.tile([128, CCH], f32, tag="gs")
        nc.vector.tensor_mul(out=gscale, in0=rstd, in1=gnw_all)
        t4 = gnp.tile([128, CCH], f32, tag="t4")
        nc.vector.tensor_mul(out=t4, in0=gst[:, :, 0], in1=gscale)
        gbias = gnp.tile([128, CCH], f32, tag="gb")
        nc.vector.tensor_sub(out=gbias, in0=gnb_all, in1=t4)

        # ---- xn in fp8, chunk-major layout [128, cc, N] (Pool engine) ----
        xn8 = xnp.tile([128, CCH, N], f8, tag="xn8")
        for cc in range(CCH):
            nc.gpsimd.tensor_scalar(
                out=xn8[:, cc, :],
                in0=x_sb[cc],
                scalar1=gscale[:, cc : cc + 1],
                scalar2=gbias[:, cc : cc + 1],
                op0=OP.mult,
                op1=OP.add,
            )

        # ---- qkv matmuls (fp8 DR, K=128-class = full speed) ----
        sc = float(HD**-0.5)
        # q/k chunks: stationary w pairs, moving xn pairs -> out [128ch, n],
        # evicted to bf16 (scores run as plain bf16 K=64 matmuls: K=32 DR
        # runs at half rate on this silicon, K>=64 at full rate).
        q8 = []
        k8 = []
        for oc in range(CCH):
            for which in ("q", "k"):
                base = 0 if which == "q" else C
                ps = ps_s.tile([128, N], f32, tag="s")
                for nj in range(2):
                    for kc in range(2):
                        nc.tensor.matmul(
                            ps[:, nj * 512 : (nj + 1) * 512],
                            lhsT=qkvw_sb[kc][:, :, base + oc * 128 : base + (oc + 1) * 128],
                            rhs=xn8[:, 2 * kc : 2 * kc + 2, nj * 512 : (nj + 1) * 512],
                            start=(kc == 0),
                            stop=(kc == 1),
                            perf_mode=DR,
                        )
                t = qkp.tile([128, N], bf16, tag=f"{which}{oc}")
                if which == "q":
                    nc.scalar.activation(
                        out=t, in_=ps, func=AF.Identity, bias=qb_sb[oc], scale=sc / WS
                    )
                    q8.append(t)
                else:
                    nc.scalar.activation(
                        out=t, in_=ps, func=AF.Identity, bias=kb_sb[oc], scale=1.0 / WS
                    )
                    k8.append(t)

        # ---- scores + exp + denominators (+ vT blocks interleaved) ----
        a8 = ap_pool.tile([128, NH, NH, N], f8, tag="a8")  # (mb, h, n)
        vt8 = vtp.tile([128, NH, NH, HD], f8, tag="vt8")  # (mb, h, d)
        psD = ps_d.tile([8, N], f32, tag="den")
        exp_idx = 0

        def emit_scores(h):
            nonlocal exp_idx
            oc, lo = h // 2, 64 * (h % 2)
            for mb in range(8):
                ps = ps_s.tile([128, N], f32, tag="s")
                for nj in range(2):
                    nc.tensor.matmul(
                        ps[:, nj * 512 : (nj + 1) * 512],
                        lhsT=k8[oc][lo : lo + 64, mb * 128 : (mb + 1) * 128],
                        rhs=q8[oc][lo : lo + 64, nj * 512 : (nj + 1) * 512],
                        start=True,
                        stop=True,
                    )
                dst = a8[:, mb, h, :]
                kind = EXP_PATTERN[exp_idx % len(EXP_PATTERN)]
                if kind == "A":
                    nc.scalar.activation(
                        out=dst, in_=ps, func=AF.Exp, bias=ebias_sb, scale=1.0
                    )
                elif kind == "U":
                    # i8 = round(S*8*log2e - corr*8); uint8 clamps both ends
                    nc.vector.tensor_scalar(
                        out=dst.bitcast(u8),
                        in0=ps,
                        scalar1=8.0 * LOG2E,
                        scalar2=-8.0 * BITEXP_CORR,
                        op0=OP.mult,
                        op1=OP.add,
                    )
                else:
                    # i16 = round(S*128*log2e + (127-ESH-corr)*128); bits = bf16 2^y
                    t16 = tmpp.tile([128, N], i16, tag="t16")
                    nc.vector.tensor_scalar(
                        out=t16,
                        in0=ps,
                        scalar1=128.0 * LOG2E,
                        scalar2=128.0 * (127.0 - ESH - BITEXP_CORR),
                        op0=OP.mult,
                        op1=OP.add,
                    )
                    nc.gpsimd.tensor_scalar(
                        out=dst,
                        in0=t16.bitcast(bf16),
                        scalar1=1.0,
                        scalar2=0.0,
                        op0=OP.mult,
                        op1=OP.add,
                    )
                exp_idx += 1

        def emit_vt(mb):
            ps = ps_s.tile([128, N], f32, tag="s")
            for kc in range(2):
                nc.tensor.matmul(
                    ps[:, 0:512],
                    lhsT=xn8[:, 2 * kc : 2 * kc + 2, mb * 128 : (mb + 1) * 128],
                    rhs=qkvw_sb[kc][:, :, 2 * C : 3 * C],
                    start=(kc == 0),
                    stop=(kc == 1),
                    perf_mode=DR,
                )
            nc.scalar.activation(
                out=vt8[:, mb, :, :],
                in_=ps[:, 0:512].rearrange("p (h d) -> p h d", h=NH),
                func=AF.Copy,
                scale=1.0 / WS,
            )

        def emit_denom(h):
            for j in range(4):
                for nj in range(2):
                    nc.tensor.matmul(
                        psD[:, nj * 512 : (nj + 1) * 512],
                        lhsT=dsel_sb[:, :, h, :],
                        rhs=a8[:, 2 * j : 2 * j + 2, h, nj * 512 : (nj + 1) * 512],
                        start=(h == 0 and j == 0),
                        stop=(h == 7 and j == 3),
                        perf_mode=DR,
                    )

        emit_scores(0)
        for mb in range(8):
            emit_vt(mb)
        for h in range(1, NH):
            emit_scores(h)
            emit_denom(h - 1)
        emit_denom(NH - 1)

        # ---- reciprocals: 1/d = exp(-ln d), broadcast via DRAM bounce ----
        lnd = recp.tile([8, N], f32, tag="lnd")
        nc.scalar.activation(out=lnd, in_=psD, func=AF.Ln, bias=zero_sb[0:8])
        recb = recp.tile([8, N], bf16, tag="recb")
        nc.scalar.activation(out=recb, in_=lnd, func=AF.Exp, scale=-1.0, bias=zero_sb[0:8])
        rec_dram = dramp.tile([8, N], bf16, tag="recd")
        nc.sync.dma_start(out=rec_dram, in_=recb)
        rec_b = recp.tile([64, NH, N], bf16, tag="recbc")
        for h in range(NH):
            row = rec_dram[h : h + 1, :]
            bcast = bass.AP(
                tensor=row.tensor,
                offset=row.offset,
                ap=[[0, 64]] + [list(x) for x in row.ap[1:]],
            )
            nc.sync.dma_start(out=rec_b[:, h, :], in_=bcast)

        # ---- AV (fp8 DR) + fused division eviction ----
        o8 = o8p.tile([128, 2, 2, N], f8, tag="o8")  # (kc, i, n); ch = kc*256+i*128+p
        for h in range(NH):
            for nj in range(2):
                av = ps_av.tile([64, 512], f32, tag="av")
                for j in range(4):
                    nc.tensor.matmul(
                        av,
                        lhsT=vt8[:, 2 * j : 2 * j + 2, h, :],
                        rhs=a8[:, 2 * j : 2 * j + 2, h, nj * 512 : (nj + 1) * 512],
                        start=(j == 0),
                        stop=(j == 3),
                        perf_mode=DR,
                    )
                nc.vector.tensor_tensor(
                    out=o8[
                        64 * (h % 2) : 64 * (h % 2) + 64,
                        h // 4,
                        (h % 4) // 2,
                        nj * 512 : (nj + 1) * 512,
                    ],
                    in0=av,
                    in1=rec_b[:, h, nj * 512 : (nj + 1) * 512],
                    op=OP.mult,
                )

        # ---- xpb (residual + folded proj/v biases) ----
        xpb_sb = []
        for cc in range(CCH):
            t = xpbp.tile([128, N], f32, tag=f"xpb{cc}")
            nc.sync.dma_start(out=t, in_=xpb_d[cc])
            xpb_sb.append(t)

        # ---- proj (fp8 DR) + residual ----
        for oc in range(CCH):
            ot = outp.tile([128, N], f32, tag="ot")
            for nj in range(2):
                ps = ps_s.tile([128, N], f32, tag="s")
                for kc in range(2):
                    nc.tensor.matmul(
                        ps[:, nj * 512 : (nj + 1) * 512],
                        lhsT=pw_sb[kc][:, :, oc * 128 : (oc + 1) * 128],
                        rhs=o8[:, kc, :, nj * 512 : (nj + 1) * 512],
                        start=(kc == 0),
                        stop=(kc == 1),
                        perf_mode=DR,
                    )
                nc.vector.scalar_tensor_tensor(
                    out=ot[:, nj * 512 : (nj + 1) * 512],
                    in0=ps[:, nj * 512 : (nj + 1) * 512],
                    scalar=1.0 / WS,
                    in1=xpb_sb[oc][:, nj * 512 : (nj + 1) * 512],
                    op0=OP.mult,
                    op1=OP.add,
                )
            nc.sync.dma_start(out=out_d[oc], in_=ot)

    _split_lw_waits(nc)
    return nc


def _split_lw_waits(nc):
    """This walrus build accepts only ONE sync-wait command per engine
    instruction; Tile can attach several. Hoist each excess wait onto its own
    pure sem-wait instruction inserted just before, in queue order."""
    from concourse import mybir

    for blk in nc.m.functions[0].blocks:
        out = []
        for inst in blk.instructions:
            si = getattr(inst, "sync_info", None)
            if (
                si is not None
                and si.on_wait
                and len(si.on_wait) > 1
                and type(inst).__name__ != "InstEventSemaphore"
            ):
                waits = list(si.on_wait)
                for j, w in enumerate(waits[:-1]):
                    sem = mybir.InstEventSemaphore(
                        name=f"{inst.name}_wsplit{j}",
                        engine=inst.engine,
                        ins=[],
                        outs=[],
                        sync_info=mybir.SyncInfo(on_wait=[w], on_update=[]),
                    )
                    out.append(sem)
                inst.sync_info = mybir.SyncInfo(
                    on_wait=waits[-1:], on_update=list(si.on_update or [])
                )
            out.append(inst)
        blk.instructions = out


def _f8(x, s=1.0):
    return np.ascontiguousarray(
        np.clip(x * s, -240.0, 240.0).astype(ml_dtypes.float8_e4m3)
    )


def kernel(x, gn_w, gn_b, qkv_w, qkv_b, proj_w, proj_b):
    from concourse.bass_utils import run_bass_kernel_spmd

    B = x.shape[0]
    assert B == NCORES
    if "nc" not in _CACHE:
        _CACHE["nc"] = _build_program()
    nc = _CACHE["nc"]

    x = np.asarray(x, dtype=np.float32)
    qkv_w = np.asarray(qkv_w, dtype=np.float32)
    qkv_b = np.asarray(qkv_b, dtype=np.float32)
    proj_w = np.asarray(proj_w, dtype=np.float32)
    proj_b = np.asarray(proj_b, dtype=np.float32)

    xf = np.ascontiguousarray(x.reshape(B, CCH, 128, N))
    # qkv weights in DR layout: [kc, cin128, i, cout], cin = kc*256+i*128+p
    wt = qkv_w.T.reshape(2, 2, 128, 3 * C)  # [kc, i, p, cout]
    qkvw8 = _f8(np.transpose(wt, (0, 2, 1, 3)), WS)  # [kc, p, i, cout]
    pwt = proj_w.T.reshape(2, 2, 128, C)
    pw8 = _f8(np.transpose(pwt, (0, 2, 1, 3)), WS)
    sc = HD**-0.5
    qb = np.ascontiguousarray((qkv_b[0:C] * sc).reshape(CCH, 128, 1))
    kb = np.ascontiguousarray(qkv_b[C : 2 * C].reshape(CCH, 128, 1))
    vb = qkv_b[2 * C : 3 * C]
    pb_eff = proj_b + proj_w @ vb
    xpb = np.ascontiguousarray(
        xf + pb_eff.reshape(CCH, 128, 1)[None, :, :, :]
    )
    gnw = np.ascontiguousarray(np.asarray(gn_w, np.float32).reshape(CCH, 128).T)
    gnb = np.ascontiguousarray(np.asarray(gn_b, np.float32).reshape(CCH, 128).T)
    mask = np.zeros((128, 128), dtype=np.float32)
    for g in range(2):
        mask[g * 64 : (g + 1) * 64, g * 64 : (g + 1) * 64] = 1.0 / 64.0
    dsel = np.zeros((128, 2, NH, NH), dtype=np.float32)
    for h in range(NH):
        dsel[:, :, h, h] = 1.0
    dsel = _f8(dsel)

    in_maps = []
    for i in range(NCORES):
        in_maps.append(
            {
                "x": xf[i],
                "xpb": xpb[i],
                "qkvw8": qkvw8,
                "pw8": pw8,
                "qb": qb,
                "kb": kb,
                "gnw": gnw,
                "gnb": gnb,
                "gn_mask": mask,
                "dsel": dsel,
            }
        )

    tmpdir = os.environ.get("BASS_TMPDIR")
    if tmpdir:
        os.makedirs(tmpdir, exist_ok=True)
    res = run_bass_kernel_spmd(
        nc,
        in_maps,
        list(range(NCORES)),
        trace=bool(os.environ.get("BASS_TRACE")),
        tmpdir=tmpdir,
    )
    LAST["exec_time_ns"] = res.exec_time_ns
    LAST["results"] = res
    out = np.stack([res.results[i]["out"] for i in range(NCORES)], axis=0)
    return out.reshape(B, C, 32, 32).astype(x.dtype)


# revision 11
# speedup vs baseline: 1.4224x; 1.1605x over previous
"""AttentionBlock kernel for 8 Trainium2 NeuronCores.

Reference op: GroupNorm(8 groups) -> 1x1 conv qkv -> 8-head attention over
1024 spatial positions -> 1x1 conv proj -> residual.   Shapes (full):
x [8, 512, 32, 32]; qkv_w [1536, 512]; proj_w [512, 512].

Sharding: pure data-parallel over batch - one batch element per core.

Design notes (v2, fp8 DoubleRow):
  - PE matmul cost is output-columns x cycles/row, independent of the
    contraction depth; fp8 DoubleRow (0.5 cyc/row) doubles throughput by
    packing two 128-deep contraction groups per instruction.  qkv, scores,
    AV and proj all run as fp8e4 DR; weights are scaled x32 into fp8 range
    (TRN e4m3 max is +-240) and unscaled at PSUM eviction.
  - Scores per head contract over d=64, split as two DR groups of 32; q/k
    are evicted to fp8 then DMA-shuffled into [32, 2, n] pair layout.
  - exp() runs with a 2^-7 shift folded into its bias so probs fit fp8;
    softmax is shift-invariant.  The exp work is split between the ACT
    engine (exact spline exp) and the DVE (fp32->uint8 bit-trick that
    directly materializes e4m3-encoded 2^y, ~5% per-element, diluted by
    mixing with exact tiles across m-chunks).
  - Denominators come from a dedicated ones-matmul over the fp8 prob tiles
    (accumulated into one [8, n] PSUM tile via one-hot stationaries) so
    reciprocals are ready before AV completes; 1/d = exp(-ln d) on ACT
    reuses the already-loaded natural_log_exp table.
  - The division is fused into the AV PSUM eviction (one tensor_tensor
    against a DMA-broadcast reciprocal tile).
  - v-bias folds through softmax+proj into a host-side residual term
    (xpb = x + proj_b + proj_w @ v_bias); q/k biases fold into the
    per-partition bias of their PSUM evictions.
"""

import os

import numpy as np
import ml_dtypes

NCORES = 8
C = 512
N = 1024  # 32*32 spatial
NH = 8
HD = 64  # head dim
CCH = 4  # channel chunks of 128
EPS = 1e-5
WS = 32.0  # fp8 weight scale
ESH = 7.0  # exp shift (probs scaled 2^-ESH)
LOG2E = 1.4426950408889634

# exp engine split, pattern over (h, mb) tiles with period 8:
#   'A' = ACT exact spline exp -> fp8
#   'U' = DVE fp32->uint8 bit-trick (1 op, ~5%/elem, keep rare)
#   'C' = DVE fp32->int16 bf16 bit-trick + Pool bf16->fp8 convert (accurate)
EXP_PATTERN = os.environ.get("EXP_PATTERN", "AACAUACA")
BITEXP_CORR = 0.0437  # linear-interp exp2 centering

_CACHE = {}
LAST = {"exec_time_ns": None, "results": None}


def _build_program():
    import concourse.bass as bass
    import concourse.tile as tile
    from concourse import mybir

    f32 = mybir.dt.float32
    bf16 = mybir.dt.bfloat16
    f8 = mybir.dt.float8e4
    u8 = mybir.dt.uint8
    i16 = mybir.dt.int16
    AF = mybir.ActivationFunctionType
    OP = mybir.AluOpType
    DR = mybir.MatmulPerfMode.DoubleRow

    nc = bass.Bass()

    # ---- DRAM parameters (per core). Host pre-reshapes/pre-scales. ----
    x_d = nc.declare_dram_parameter("x", [CCH, 128, N], f32, isOutput=False)
    xpb_d = nc.declare_dram_parameter("xpb", [CCH, 128, N], f32, isOutput=False)
    qkvw_d = nc.declare_dram_parameter("qkvw8", [2, 128, 2, 3 * C], f8, isOutput=False)
    pw_d = nc.declare_dram_parameter("pw8", [2, 128, 2, C], f8, isOutput=False)
    qb_d = nc.declare_dram_parameter("qb", [CCH, 128, 1], f32, isOutput=False)
    kb_d = nc.declare_dram_parameter("kb", [CCH, 128, 1], f32, isOutput=False)
    gnw_d = nc.declare_dram_parameter("gnw", [128, CCH], f32, isOutput=False)
    gnb_d = nc.declare_dram_parameter("gnb", [128, CCH], f32, isOutput=False)
    mask_d = nc.declare_dram_parameter("gn_mask", [128, 128], f32, isOutput=False)
    dsel_d = nc.declare_dram_parameter("dsel", [128, 2, NH, NH], f8, isOutput=False)
    out_d = nc.declare_dram_parameter("out", [CCH, 128, N], f32, isOutput=True)

    from contextlib import ExitStack

    with (
        nc.allow_low_precision(reason="fp8 double-row matmuls within tolerance"),
        tile.TileContext(nc) as tc,
        ExitStack() as ctx,
    ):
        consts = ctx.enter_context(tc.tile_pool(name="consts", bufs=1))
        xp = ctx.enter_context(tc.tile_pool(name="xp", bufs=1))
        xpbp = ctx.enter_context(tc.tile_pool(name="xpbp", bufs=1))
        wp = ctx.enter_context(tc.tile_pool(name="wp", bufs=1))
        xnp = ctx.enter_context(tc.tile_pool(name="xnp", bufs=1))
        qkp = ctx.enter_context(tc.tile_pool(name="qkp", bufs=1))
        vtp = ctx.enter_context(tc.tile_pool(name="vtp", bufs=1))
        ap_pool = ctx.enter_context(tc.tile_pool(name="ap", bufs=1))
        gnp = ctx.enter_context(tc.tile_pool(name="gnp", bufs=1))
        o8p = ctx.enter_context(tc.tile_pool(name="o8p", bufs=1))
        recp = ctx.enter_context(tc.tile_pool(name="recp", bufs=1))
        tmpp = ctx.enter_context(tc.tile_pool(name="tmpp", bufs=2))
        outp = ctx.enter_context(tc.tile_pool(name="outp", bufs=2))
        dramp = ctx.enter_context(tc.tile_pool(name="dramp", bufs=1, space="DRAM"))
        # PSUM: shared [128,1024] x3 (6 banks, scores/qkv/AV/proj) + denom (2)
        ps_s = ctx.enter_context(tc.tile_pool(name="ps_s", bufs=3, space="PSUM"))
        ps_d = ctx.enter_context(tc.tile_pool(name="ps_d", bufs=1, space="PSUM"))

        # ---- load x first (gates GN on the critical path) ----
        x_sb = []
        for cc in range(CCH):
            t = xp.tile([128, N], f32, tag=f"x{cc}")
            nc.sync.dma_start(out=t, in_=x_d[cc])
            x_sb.append(t)

        # ---- constants ----
        mask_sb = consts.tile([128, 128], f32, tag="mask")
        nc.sync.dma_start(out=mask_sb, in_=mask_d[:, :])
        gnw_all = consts.tile([128, CCH], f32, tag="gnw")
        nc.sync.dma_start(out=gnw_all, in_=gnw_d[:, :])
        gnb_all = consts.tile([128, CCH], f32, tag="gnb")
        nc.sync.dma_start(out=gnb_all, in_=gnb_d[:, :])
        qb_sb = []
        kb_sb = []
        for cc in range(CCH):
            t = consts.tile([128, 1], f32, tag=f"qb{cc}")
            nc.sync.dma_start(out=t, in_=qb_d[cc])
            qb_sb.append(t)
            t = consts.tile([128, 1], f32, tag=f"kb{cc}")
            nc.sync.dma_start(out=t, in_=kb_d[cc])
            kb_sb.append(t)
        dsel_sb = consts.tile([128, 2, NH, NH], f8, tag="dsel")
        nc.sync.dma_start(out=dsel_sb, in_=dsel_d[:, :, :, :])
        eps_sb = consts.tile([128, 1], f32, tag="eps")
        nc.vector.memset(eps_sb, EPS)
        zero_sb = consts.tile([128, 1], f32, tag="zero")
        nc.vector.memset(zero_sb, 0.0)
        ebias_sb = consts.tile([128, 1], f32, tag="ebias")
        nc.vector.memset(ebias_sb, -ESH * 0.6931471805599453)

        # ---- weights ----
        qkvw_sb = []
        for kc in range(2):
            t = wp.tile([128, 2, 3 * C], f8, tag=f"qw{kc}")
            nc.sync.dma_start(out=t, in_=qkvw_d[kc])
            qkvw_sb.append(t)
        pw_sb = []
        for kc in range(2):
            t = wp.tile([128, 2, C], f8, tag=f"pw{kc}")
            nc.sync.dma_start(out=t, in_=pw_d[kc])
            pw_sb.append(t)

        # ---- GroupNorm stats (as v1: bn_stats + block-diag mask matmul) ----
        mv_all = gnp.tile([128, CCH, 2], f32, tag="mv")
        for cc in range(CCH):
            stats = gnp.tile([128, 2, 6], f32, tag=f"st{cc}")
            for sg in range(2):
                nc.vector.bn_stats(
                    out=stats[:, sg, :], in_=x_sb[cc][:, sg * 512 : (sg + 1) * 512]
                )
            nc.vector.bn_aggr(out=mv_all[:, cc, :], in_=stats)
        st2 = gnp.tile([128, CCH, 2], f32, tag="s2")
        nc.vector.tensor_copy(out=st2[:, :, 0], in_=mv_all[:, :, 0])
        mean_sq = gnp.tile([128, CCH], f32, tag="msq")
        nc.vector.tensor_mul(out=mean_sq, in0=mv_all[:, :, 0], in1=mv_all[:, :, 0])
        nc.vector.tensor_add(out=st2[:, :, 1], in0=mv_all[:, :, 1], in1=mean_sq)
        ps_gn = ps_s.tile([128, N], f32, tag="s")
        ps_st = ps_gn[:, 0 : CCH * 2]
        nc.tensor.matmul(
            ps_st,
            lhsT=mask_sb,
            rhs=st2.rearrange("p c two -> p (c two)"),
            start=True,
            stop=True,
        )
        gst = gnp.tile([128, CCH, 2], f32, tag="gst")
        nc.vector.tensor_copy(
            out=gst, in_=ps_st.rearrange("p (c two) -> p c two", two=2)
        )
        gm2 = gnp.tile([128, CCH], f32, tag="g2")
        nc.vector.tensor_mul(out=gm2, in0=gst[:, :, 0], in1=gst[:, :, 0])
        gvar = gnp.tile([128, CCH], f32, tag="gv")
        nc.vector.tensor_sub(out=gvar, in0=gst[:, :, 1], in1=gm2)
        # rstd = exp(-0.5*ln(var+eps)); warms the natural_log_exp ACT table.
        lnv = gnp.tile([128, CCH], f32, tag="lnv")
        nc.scalar.activation(out=lnv, in_=gvar, func=AF.Ln, bias=eps_sb)
        rstd = gnp.tile([128, CCH], f32, tag="rstd")
        nc.scalar.activation(out=rstd, in_=lnv, func=AF.Exp, scale=-0.5, bias=zero_sb)
        gscale = gnp.tile([128, CCH], f32, tag="gs")
        nc.vector.tensor_mul(out=gscale, in0=rstd, in1=gnw_all)
        t4 = gnp.tile([128, CCH], f32, tag="t4")
        nc.vector.tensor_mul(out=t4, in0=gst[:, :, 0], in1=gscale)
        gbias = gnp.tile([128, CCH], f32, tag="gb")
        nc.vector.tensor_sub(out=gbias, in0=gnb_all, in1=t4)

        # ---- xn in fp8, chunk-major layout [128, cc, N] (Pool engine) ----
        xn8 = xnp.tile([128, CCH, N], f8, tag="xn8")
        for cc in range(CCH):
            eng = nc.vector if cc % 2 == 0 else nc.gpsimd
            eng.tensor_scalar(
                out=xn8[:, cc, :],
                in0=x_sb[cc],
                scalar1=gscale[:, cc : cc + 1],
                scalar2=gbias[:, cc : cc + 1],
                op0=OP.mult,
                op1=OP.add,
            )

        # ---- qkv matmuls (fp8 DR, K=128-class = full speed) ----
        sc = float(HD**-0.5)
        # q/k chunks: stationary w pairs, moving xn pairs -> out [128ch, n],
        # evicted to bf16 (scores run as plain bf16 K=64 matmuls: K=32 DR
        # runs at half rate on this silicon, K>=64 at full rate).
        q8 = []
        k8 = []
        for oc in range(CCH):
            for which in ("q", "k"):
                base = 0 if which == "q" else C
                ps = ps_s.tile([128, N], f32, tag="s")
                for nj in range(2):
                    for kc in range(2):
                        nc.tensor.matmul(
                            ps[:, nj * 512 : (nj + 1) * 512],
                            lhsT=qkvw_sb[kc][:, :, base + oc * 128 : base + (oc + 1) * 128],
                            rhs=xn8[:, 2 * kc : 2 * kc + 2, nj * 512 : (nj + 1) * 512],
                            start=(kc == 0),
                            stop=(kc == 1),
                            perf_mode=DR,
                        )
                t = qkp.tile([128, N], bf16, tag=f"{which}{oc}")
                if which == "q":
                    nc.scalar.activation(
                        out=t, in_=ps, func=AF.Identity, bias=qb_sb[oc], scale=sc / WS
                    )
                    q8.append(t)
                else:
                    nc.scalar.activation(
                        out=t, in_=ps, func=AF.Identity, bias=kb_sb[oc], scale=1.0 / WS
                    )
                    k8.append(t)

        # ---- scores + exp + denominators (+ vT blocks interleaved) ----
        a8 = ap_pool.tile([128, NH, NH, N], f8, tag="a8")  # (mb, h, n)
        vt8 = vtp.tile([128, NH, NH, HD], f8, tag="vt8")  # (mb, h, d)
        psD = ps_d.tile([8, N], f32, tag="den")
        exp_idx = 0

        def emit_exp(kind, ps, dst):
            if kind == "A":
                nc.scalar.activation(
                    out=dst, in_=ps, func=AF.Exp, bias=ebias_sb, scale=1.0
                )
            elif kind == "U":
                # i8 = round(S*8*log2e - corr*8); uint8 clamps both ends
                nc.vector.tensor_scalar(
                    out=dst.bitcast(u8),
                    in0=ps,
                    scalar1=8.0 * LOG2E,
                    scalar2=-8.0 * BITEXP_CORR,
                    op0=OP.mult,
                    op1=OP.add,
                )
            else:
                # i16 = round(S*128*log2e + (127-ESH-corr)*128); bits = bf16 2^y
                t16 = tmpp.tile([128, N], i16, tag="t16")
                nc.vector.tensor_scalar(
                    out=t16,
                    in0=ps,
                    scalar1=128.0 * LOG2E,
                    scalar2=128.0 * (127.0 - ESH - BITEXP_CORR),
                    op0=OP.mult,
                    op1=OP.add,
                )
                nc.gpsimd.tensor_scalar(
                    out=dst,
                    in0=t16.bitcast(bf16),
                    scalar1=1.0,
                    scalar2=0.0,
                    op0=OP.mult,
                    op1=OP.add,
                )

        def emit_scores(h):
            nonlocal exp_idx
            oc, lo = h // 2, 64 * (h % 2)
            for mb in range(8):
                ps = ps_s.tile([128, N], f32, tag="s")
                for nj in range(2):
                    nc.tensor.matmul(
                        ps[:, nj * 512 : (nj + 1) * 512],
                        lhsT=k8[oc][lo : lo + 64, mb * 128 : (mb + 1) * 128],
                        rhs=q8[oc][lo : lo + 64, nj * 512 : (nj + 1) * 512],
                        start=True,
                        stop=True,
                    )
                dst = a8[:, mb, h, :]
                kind = EXP_PATTERN[exp_idx % len(EXP_PATTERN)]
                emit_exp(kind, ps, dst)
                exp_idx += 1

        def emit_vt(mb):
            ps = ps_s.tile([128, N], f32, tag="s")
            for kc in range(2):
                nc.tensor.matmul(
                    ps[:, 0:512],
                    lhsT=xn8[:, 2 * kc : 2 * kc + 2, mb * 128 : (mb + 1) * 128],
                    rhs=qkvw_sb[kc][:, :, 2 * C : 3 * C],
                    start=(kc == 0),
                    stop=(kc == 1),
                    perf_mode=DR,
                )
            nc.scalar.activation(
                out=vt8[:, mb, :, :],
                in_=ps[:, 0:512].rearrange("p (h d) -> p h d", h=NH),
                func=AF.Copy,
                scale=1.0 / WS,
            )

        def emit_denom(h, j):
            for nj in range(2):
                nc.tensor.matmul(
                    psD[:, nj * 512 : (nj + 1) * 512],
                    lhsT=dsel_sb[:, :, h, :],
                    rhs=a8[:, 2 * j : 2 * j + 2, h, nj * 512 : (nj + 1) * 512],
                    start=(h == 0 and j == 0),
                    stop=(h == 7 and j == 3),
                    perf_mode=DR,
                )

        emit_scores(0)
        for mb in range(8):
            emit_vt(mb)
        for h in range(1, NH):
            # spread the previous head's denom instructions between this
            # head's score tiles so a blocked scores matmul never sits in
            # front of runnable work on the in-order PE queue.
            oc, lo = h // 2, 64 * (h % 2)
            for mb in range(8):
                ps = ps_s.tile([128, N], f32, tag="s")
                for nj in range(2):
                    nc.tensor.matmul(
                        ps[:, nj * 512 : (nj + 1) * 512],
                        lhsT=k8[oc][lo : lo + 64, mb * 128 : (mb + 1) * 128],
                        rhs=q8[oc][lo : lo + 64, nj * 512 : (nj + 1) * 512],
                        start=True,
                        stop=True,
                    )
                if mb % 2 == 1:
                    emit_denom(h - 1, mb // 2)
                dst = a8[:, mb, h, :]
                kind = EXP_PATTERN[exp_idx % len(EXP_PATTERN)]
                emit_exp(kind, ps, dst)
                exp_idx += 1
        for j in range(4):
            emit_denom(NH - 1, j)

        # ---- reciprocals: 1/d = exp(-ln d), broadcast via DRAM bounce ----
        lnd = recp.tile([8, N], f32, tag="lnd")
        nc.scalar.activation(out=lnd, in_=psD, func=AF.Ln, bias=zero_sb[0:8])
        recb = recp.tile([8, N], bf16, tag="recb")
        nc.scalar.activation(out=recb, in_=lnd, func=AF.Exp, scale=-1.0, bias=zero_sb[0:8])
        rec_dram = dramp.tile([8, N], bf16, tag="recd")
        nc.sync.dma_start(out=rec_dram, in_=recb)
        rec_b = recp.tile([64, NH, N], bf16, tag="recbc")
        for h in range(NH):
            row = rec_dram[h : h + 1, :]
            bcast = bass.AP(
                tensor=row.tensor,
                offset=row.offset,
                ap=[[0, 64]] + [list(x) for x in row.ap[1:]],
            )
            nc.sync.dma_start(out=rec_b[:, h, :], in_=bcast)

        # ---- AV (fp8 DR) + fused division eviction ----
        o8 = o8p.tile([128, 2, 2, N], f8, tag="o8")  # (kc, i, n); ch = kc*256+i*128+p
        for h in range(NH):
            for nj in range(2):
                av_t = ps_s.tile([128, N], f32, tag="s")
                av = av_t[0:64, 0:512]
                for j in range(4):
                    nc.tensor.matmul(
                        av,
                        lhsT=vt8[:, 2 * j : 2 * j + 2, h, :],
                        rhs=a8[:, 2 * j : 2 * j + 2, h, nj * 512 : (nj + 1) * 512],
                        start=(j == 0),
                        stop=(j == 3),
                        perf_mode=DR,
                    )
                nc.vector.tensor_tensor(
                    out=o8[
                        64 * (h % 2) : 64 * (h % 2) + 64,
                        h // 4,
                        (h % 4) // 2,
                        nj * 512 : (nj + 1) * 512,
                    ],
                    in0=av,
                    in1=rec_b[:, h, nj * 512 : (nj + 1) * 512],
                    op=OP.mult,
                )

        # ---- xpb (residual + folded proj/v biases) ----
        xpb_sb = []
        for cc in range(CCH):
            t = xpbp.tile([128, N], f32, tag=f"xpb{cc}")
            nc.sync.dma_start(out=t, in_=xpb_d[cc])
            xpb_sb.append(t)

        # ---- proj (fp8 DR) + residual ----
        for oc in range(CCH):
            ot = outp.tile([128, N], f32, tag="ot")
            for nj in range(2):
                ps = ps_s.tile([128, N], f32, tag="s")
                for kc in range(2):
                    nc.tensor.matmul(
                        ps[:, nj * 512 : (nj + 1) * 512],
                        lhsT=pw_sb[kc][:, :, oc * 128 : (oc + 1) * 128],
                        rhs=o8[:, kc, :, nj * 512 : (nj + 1) * 512],
                        start=(kc == 0),
                        stop=(kc == 1),
                        perf_mode=DR,
                    )
                nc.vector.scalar_tensor_tensor(
                    out=ot[:, nj * 512 : (nj + 1) * 512],
                    in0=ps[:, nj * 512 : (nj + 1) * 512],
                    scalar=1.0 / WS,
                    in1=xpb_sb[oc][:, nj * 512 : (nj + 1) * 512],
                    op0=OP.mult,
                    op1=OP.add,
                )
            nc.sync.dma_start(out=out_d[oc], in_=ot)

    _split_lw_waits(nc)
    return nc


def _split_lw_waits(nc):
    """This walrus build accepts only ONE sync-wait command per engine
    instruction; Tile can attach several. Hoist each excess wait onto its own
    pure sem-wait instruction inserted just before, in queue order."""
    from concourse import mybir

    for blk in nc.m.functions[0].blocks:
        out = []
        for inst in blk.instructions:
            si = getattr(inst, "sync_info", None)
            if (
                si is not None
                and si.on_wait
                and len(si.on_wait) > 1
                and type(inst).__name__ != "InstEventSemaphore"
            ):
                waits = list(si.on_wait)
                for j, w in enumerate(waits[:-1]):
                    sem = mybir.InstEventSemaphore(
                        name=f"{inst.name}_wsplit{j}",
                        engine=inst.engine,
                        ins=[],
                        outs=[],
                        sync_info=mybir.SyncInfo(on_wait=[w], on_update=[]),
                    )
                    out.append(sem)
                inst.sync_info = mybir.SyncInfo(
                    on_wait=waits[-1:], on_update=list(si.on_update or [])
                )
            out.append(inst)
        blk.instructions = out


def _f8(x, s=1.0):
    return np.ascontiguousarray(
        np.clip(x * s, -240.0, 240.0).astype(ml_dtypes.float8_e4m3)
    )


def kernel(x, gn_w, gn_b, qkv_w, qkv_b, proj_w, proj_b):
    from concourse.bass_utils import run_bass_kernel_spmd

    B = x.shape[0]
    assert B == NCORES
    if "nc" not in _CACHE:
        _CACHE["nc"] = _build_program()
    nc = _CACHE["nc"]

    x = np.asarray(x, dtype=np.float32)
    qkv_w = np.asarray(qkv_w, dtype=np.float32)
    qkv_b = np.asarray(qkv_b, dtype=np.float32)
    proj_w = np.asarray(proj_w, dtype=np.float32)
    proj_b = np.asarray(proj_b, dtype=np.float32)

    xf = np.ascontiguousarray(x.reshape(B, CCH, 128, N))
    # qkv weights in DR layout: [kc, cin128, i, cout], cin = kc*256+i*128+p
    wt = qkv_w.T.reshape(2, 2, 128, 3 * C)  # [kc, i, p, cout]
    qkvw8 = _f8(np.transpose(wt, (0, 2, 1, 3)), WS)  # [kc, p, i, cout]
    pwt = proj_w.T.reshape(2, 2, 128, C)
    pw8 = _f8(np.transpose(pwt, (0, 2, 1, 3)), WS)
    sc = HD**-0.5
    qb = np.ascontiguousarray((qkv_b[0:C] * sc).reshape(CCH, 128, 1))
    kb = np.ascontiguousarray(qkv_b[C : 2 * C].reshape(CCH, 128, 1))
    vb = qkv_b[2 * C : 3 * C]
    pb_eff = proj_b + proj_w @ vb
    xpb = np.ascontiguousarray(
        xf + pb_eff.reshape(CCH, 128, 1)[None, :, :, :]
    )
    gnw = np.ascontiguousarray(np.asarray(gn_w, np.float32).reshape(CCH, 128).T)
    gnb = np.ascontiguousarray(np.asarray(gn_b, np.float32).reshape(CCH, 128).T)
    mask = np.zeros((128, 128), dtype=np.float32)
    for g in range(2):
        mask[g * 64 : (g + 1) * 64, g * 64 : (g + 1) * 64] = 1.0 / 64.0
    dsel = np.zeros((128, 2, NH, NH), dtype=np.float32)
    for h in range(NH):
        dsel[:, :, h, h] = 1.0
    dsel = _f8(dsel)

    in_maps = []
    for i in range(NCORES):
        in_maps.append(
            {
                "x": xf[i],
                "xpb": xpb[i],
                "qkvw8": qkvw8,
                "pw8": pw8,
                "qb": qb,
                "kb": kb,
                "gnw": gnw,
                "gnb": gnb,
                "gn_mask": mask,
                "dsel": dsel,
            }
        )

    tmpdir = os.environ.get("BASS_TMPDIR")
    if tmpdir:
        os.makedirs(tmpdir, exist_ok=True)
    res = run_bass_kernel_spmd(
        nc,
        in_maps,
        list(range(NCORES)),
        trace=bool(os.environ.get("BASS_TRACE")),
        tmpdir=tmpdir,
    )
    LAST["exec_time_ns"] = res.exec_time_ns
    LAST["results"] = res
    out = np.stack([res.results[i]["out"] for i in range(NCORES)], axis=0)
    return out.reshape(B, C, 32, 32).astype(x.dtype)
